# revision 13
# baseline (speedup 1.0000x reference)
"""Trainium2 Bass kernel for NeuralVMEmbedding (embedding lookup + VM channel injection).

Strategy (pure data-parallel over batch; 8 cores, 4 rows of 8192 tokens each):
  - Table uploaded as bf16 (tolerance 2e-2 >> bf16's ~4e-3), output written
    bf16 and upcast to f32 on host -> halves all HBM/DMA traffic vs f32.
  - Hybrid gather, split per 1024-token group (32 groups/core):
      * NDG groups via gpsimd dma_gather (SWDGE ucode, ~8.8ns/row on Pool;
        >1024 idxs per instruction crashes the SWDGE ring - keep 1024).
        Host-permuted idx list lands tiles directly in PM layout.
      * remaining groups via PE one-hot matmul: token row broadcast with a
        K=1 outer product, one-hot built by is_equal vs per-partition iota,
        3 vocab-chunk matmuls (K=128/128/16) accumulate table rows in PSUM,
        scalar engine converts PSUM f32 -> SBUF bf16.
  - ADDR_KEY one-hot + MEM_STORE injection computed on-chip (scans on DVE,
    copy_predicated patches), identical for both gather paths.
  - Output written back with 8KB-contiguous DMA runs via the sync HWDGE queue.
"""

import sys
import numpy as np

for _p in ("/opt/trn_rl_repo",):
    if _p not in sys.path:
        sys.path.insert(0, _p)

# ---- problem constants (hardcoded per contract) ----
B, S, D, V = 32, 8192, 512, 272
NCORES = 8
RPC = B // NCORES          # rows (batch) per core = 4
P = 128                    # partitions
CPR = S // P               # columns per row in partition-major layout = 64
CTILE = 8                  # tile width in columns (CTILE*128 = 1024 tokens)
NTOK = RPC * S             # tokens per core = 32768
NG = NTOK // (P * CTILE)   # groups per core = 32
NDG = 8                    # groups gathered via dma_gather; rest via PE
IDXW = NDG * P * CTILE // 16
ADDR_KEY = 206
MEM_STORE = 455

_CACHE = {}


def _build(mhe: int):
    from concourse import bass, bacc, mybir, tile

    f32 = mybir.dt.float32
    f16 = mybir.dt.float16
    bf16 = mybir.dt.bfloat16
    i32 = mybir.dt.int32
    i16 = mybir.dt.int16
    u8 = mybir.dt.uint8
    Alu = mybir.AluOpType

    nc = bacc.Bacc(None)
    tok_d = nc.declare_dram_parameter("tok", [RPC, S], i32, isOutput=False)
    idx_d = nc.declare_dram_parameter("idx", [P, IDXW], i16, isOutput=False)
    tokr_d = nc.declare_dram_parameter("tokr", [1, NTOK], f16, isOutput=False)
    tab_d = nc.declare_dram_parameter("table", [V, D], bf16, isOutput=False)
    out_d = nc.declare_dram_parameter("out", [RPC, S, D], bf16, isOutput=True)

    with tile.TileContext(nc) as tc:
        with tc.tile_pool(name="const", bufs=1) as constp, \
             tc.tile_pool(name="pre", bufs=1) as pre, \
             tc.tile_pool(name="dramp", bufs=1, space="DRAM") as dramp, \
             tc.tile_pool(name="mainp", bufs=4) as mainp, \
             tc.tile_pool(name="condp", bufs=3) as condp, \
             tc.tile_pool(name="ohp", bufs=3) as ohp, \
             tc.tile_pool(name="bcp", bufs=2, space="PSUM") as bcp, \
             tc.tile_pool(name="outp", bufs=3, space="PSUM") as outp:

            # ---------------- constants ----------------
            iota48_i = constp.tile([P, CTILE, 3, 16], i32)
            nc.gpsimd.iota(iota48_i[:], pattern=[[0, CTILE], [0, 3], [1, 16]],
                           base=0, channel_multiplier=0)
            iota48 = constp.tile([P, CTILE, 3, 16], f32)
            nc.vector.tensor_copy(iota48[:], iota48_i[:])

            # padded to 64 in the last dim so [:, :, 0:48] slices keep a
            # 3-D access pattern matching the strided x[...] views
            ones48 = constp.tile([P, CTILE, 64], bf16)
            nc.vector.memset(ones48[:], 1.0)

            pos_i = constp.tile([P, RPC, CPR], i32)   # pos = 64*p + c (per row)
            nc.gpsimd.iota(pos_i[:], pattern=[[0, RPC], [1, CPR]], base=0,
                           channel_multiplier=CPR)
            pos_f = constp.tile([P, RPC, CPR], f32)
            nc.vector.tensor_copy(pos_f[:], pos_i[:])

            # PE-gather constants: table chunks, ones row, per-chunk iota cols
            tab1 = constp.tile([P, D], bf16)
            nc.sync.dma_start(out=tab1[:], in_=tab_d[0:128, :])
            tab2 = constp.tile([P, D], bf16)
            nc.sync.dma_start(out=tab2[:], in_=tab_d[128:256, :])
            tab3 = constp.tile([16, D], bf16)
            nc.sync.dma_start(out=tab3[:], in_=tab_d[256:272, :])
            tabs = (tab1, tab2, tab3)

            ones1 = constp.tile([1, P], f16)
            nc.vector.memset(ones1[:], 1.0)

            iotav_i = constp.tile([P, 3], i32)        # p + 128k
            nc.gpsimd.iota(iotav_i[:], pattern=[[128, 3]], base=0,
                           channel_multiplier=1)
            iotav = constp.tile([P, 3], f32)
            nc.vector.tensor_copy(iotav[:], iotav_i[:])

            tokr_sb = constp.tile([1, NTOK], f16)
            nc.sync.dma_start(out=tokr_sb[:], in_=tokr_d[:])

            # ---------------- token / idx load ----------------
            tok_i = pre.tile([P, RPC, CPR], i32)
            nc.sync.dma_start(out=tok_i[:],
                              in_=tok_d[:].rearrange("r (p c) -> p r c", p=P))
            tok_f = pre.tile([P, RPC, CPR], f32)
            nc.vector.tensor_copy(tok_f[:], tok_i[:])

            idx_sb = pre.tile([P, IDXW], i16)
            nc.sync.dma_start(out=idx_sb[:], in_=idx_d[:])

            # ---------------- scan inputs ----------------
            posp1 = pre.tile([P, RPC, CPR], f32)
            nc.vector.tensor_scalar(posp1[:], pos_f[:], 1.0, None, Alu.add)
            posm1 = pre.tile([P, RPC, CPR], f32)
            nc.vector.tensor_scalar(posm1[:], pos_f[:], 1.0, None, Alu.subtract)

            # v0 = (tok==256)*(pos+1) - 1   (CODE_START candidate positions)
            v0 = pre.tile([P, RPC, CPR], f32)
            nc.vector.scalar_tensor_tensor(v0[:], tok_f[:], 256.0, posp1[:],
                                           Alu.is_equal, Alu.mult)
            nc.vector.tensor_scalar(v0[:], v0[:], 1.0, None, Alu.subtract)

            # v1 = (tok==257)  (CODE_END seen)
            v1 = pre.tile([P, RPC, CPR], f32)
            nc.vector.tensor_scalar(v1[:], tok_f[:], 257.0, None, Alu.is_equal)

            cs = pre.tile([P, RPC, CPR], f32)
            ce = pre.tile([P, RPC, CPR], f32)

            # --- level 1: within-partition prefix max over 64-token chunks ---
            loc_cs = pre.tile([P, RPC, CPR], f32)
            loc_ce = pre.tile([P, RPC, CPR], f32)
            for r in range(RPC):
                nc.vector.tensor_tensor_scan(loc_cs[:, r, :], v0[:, r, :],
                                             v0[:, r, :], -1.0,
                                             Alu.max, Alu.bypass)
                nc.vector.tensor_tensor_scan(loc_ce[:, r, :], v1[:, r, :],
                                             v1[:, r, :], 0.0,
                                             Alu.max, Alu.bypass)

            # --- level 2: exclusive prefix max across partitions (chunks) ---
            # Collect the 8 per-partition chunk-final columns (cs rows 0-3,
            # ce rows 4-7), transpose [128, 8] -> [8, 128] via a tiny DRAM
            # round-trip, scan along the free dim, shift for exclusivity,
            # transpose back.
            NS = 2 * RPC
            f8 = pre.tile([P, NS], f32)
            for r in range(RPC):
                nc.vector.tensor_copy(f8[:, r:r + 1],
                                      loc_cs[:, r, CPR - 1:CPR])
                nc.vector.tensor_copy(f8[:, RPC + r:RPC + r + 1],
                                      loc_ce[:, r, CPR - 1:CPR])
            f8_d = dramp.tile([P, NS], f32)
            nc.sync.dma_start(out=f8_d[:], in_=f8[:])
            f8t = pre.tile([NS, P], f32)
            nc.sync.dma_start(out=f8t[:], in_=f8_d[:].rearrange("p j -> j p"))
            p8 = pre.tile([NS, P], f32)
            nc.vector.tensor_tensor_scan(p8[:], f8t[:], f8t[:], -1e30,
                                         Alu.max, Alu.bypass)
            e8t = pre.tile([NS, P], f32)
            # -1 is a neutral carry for both scans (cs values >= -1, ce >= 0)
            nc.vector.memset(e8t[:, 0:1], -1.0)
            nc.vector.tensor_copy(e8t[:, 1:P], p8[:, 0:P - 1])
            e8_d = dramp.tile([NS, P], f32)
            nc.sync.dma_start(out=e8_d[:], in_=e8t[:])
            e8 = pre.tile([P, NS], f32)
            nc.sync.dma_start(out=e8[:], in_=e8_d[:].rearrange("j p -> p j"))

            # --- combine ---
            for r in range(RPC):
                nc.vector.tensor_scalar(cs[:, r, :], loc_cs[:, r, :],
                                        e8[:, r:r + 1], None, Alu.max)
                nc.vector.tensor_scalar(ce[:, r, :], loc_ce[:, r, :],
                                        e8[:, RPC + r:RPC + r + 1], None,
                                        Alu.max)

            # ---------------- per-token derived values ----------------
            # mask = (cs >= 0) & (ce == 0) & (tok < 256)
            m3 = pre.tile([P, RPC, CPR], f32)
            nc.vector.tensor_scalar(m3[:], tok_f[:], 255.5, None, Alu.is_lt)
            m23 = pre.tile([P, RPC, CPR], f32)
            nc.vector.scalar_tensor_tensor(m23[:], ce[:], 0.5, m3[:],
                                           Alu.is_lt, Alu.mult)
            mask = pre.tile([P, RPC, CPR], f32)
            nc.vector.scalar_tensor_tensor(mask[:], cs[:], 0.0, m23[:],
                                           Alu.is_ge, Alu.mult)

            # seq_pos = max(pos - 1 - cs, 0)
            sp = pre.tile([P, RPC, CPR], f32)
            nc.vector.scalar_tensor_tensor(sp[:], cs[:], -1.0, posm1[:],
                                           Alu.mult, Alu.add)
            nc.vector.tensor_scalar(sp[:], sp[:], 0.0, None, Alu.max)

            # q = floor(sp / 5), robust to cast rounding mode:
            #   y = sp*0.2 ; q0 = int(y) ; q = q0 - (y - float(q0) < 0)
            y = pre.tile([P, RPC, CPR], f32)
            nc.vector.tensor_scalar(y[:], sp[:], 0.2, None, Alu.mult)
            q_i = pre.tile([P, RPC, CPR], i32)
            nc.vector.tensor_copy(q_i[:], y[:])
            q_f = pre.tile([P, RPC, CPR], f32)
            nc.vector.tensor_copy(q_f[:], q_i[:])
            corr = pre.tile([P, RPC, CPR], f32)
            nc.vector.tensor_tensor(corr[:], y[:], q_f[:], Alu.subtract)
            nc.vector.tensor_scalar(corr[:], corr[:], 0.0, None, Alu.is_lt)
            nc.vector.tensor_tensor(q_f[:], q_f[:], corr[:], Alu.subtract)

            # addr = sp + 3*q  (int32)
            sp_i = pre.tile([P, RPC, CPR], i32)
            nc.vector.tensor_copy(sp_i[:], sp[:])
            q_i2 = pre.tile([P, RPC, CPR], i32)
            nc.vector.tensor_copy(q_i2[:], q_f[:])
            q3 = pre.tile([P, RPC, CPR], i32)
            nc.vector.tensor_scalar(q3[:], q_i2[:], 1, None, Alu.logical_shift_left)
            nc.vector.tensor_tensor(q3[:], q3[:], q_i2[:], Alu.add)
            addr = pre.tile([P, RPC, CPR], i32)
            nc.vector.tensor_tensor(addr[:], sp_i[:], q3[:], Alu.add)

            # nibbles
            lo_i = pre.tile([P, RPC, CPR], i32)
            nc.vector.tensor_scalar(lo_i[:], addr[:], 15, None, Alu.bitwise_and)
            hi_i = pre.tile([P, RPC, CPR], i32)
            nc.vector.tensor_scalar(hi_i[:], addr[:], 4, 15,
                                    Alu.logical_shift_right, Alu.bitwise_and)
            top_i = pre.tile([P, RPC, CPR], i32)
            nc.vector.tensor_scalar(top_i[:], addr[:], 8, 15,
                                    Alu.logical_shift_right, Alu.bitwise_and)
            # masked nibbles: nib_m = nib + 16*(1-mask) -- unmasked tokens
            # get an out-of-range value (>=16) so the iota 0..15 compare in
            # the cond build never fires; kills the separate mask multiply.
            # (single allocation: same-call-site tiles alias in a bufs=1 pool)
            nm3 = pre.tile([P, 3, RPC, CPR], f32)
            for b, src_i in enumerate((lo_i, hi_i, top_i)):
                nc.vector.tensor_copy(nm3[:, b], src_i[:])
            nc.vector.tensor_scalar(nm3[:], nm3[:], 16.0, None, Alu.add)
            for b in range(3):
                nc.vector.scalar_tensor_tensor(nm3[:, b], mask[:], -16.0,
                                               nm3[:, b], Alu.mult, Alu.add)

            # cond2 = (tok == 258) & (pos < mem_history_end)
            m5 = pre.tile([P, RPC, CPR], f32)
            nc.vector.tensor_scalar(m5[:], pos_f[:], float(mhe), None, Alu.is_lt)
            cond2 = pre.tile([P, RPC, CPR], u8)
            nc.vector.scalar_tensor_tensor(cond2[:], tok_f[:], 258.0, m5[:],
                                           Alu.is_equal, Alu.mult)

            # ---------------- main gather + patch + store loop ----------------
            out_v = out_d[:].rearrange("r (p c) d -> r p c d", p=P)
            NI = P * CTILE                 # tokens per group = 1024

            def patch_and_store(x, r, csl):
                cond = condp.tile([P, CTILE, 64], u8, tag="cond")
                for b in range(3):
                    nc.vector.tensor_tensor(
                        cond[:, :, 16 * b:16 * (b + 1)],
                        iota48[:, :, b, :],
                        nm3[:, b, r, csl].to_broadcast([P, CTILE, 16]),
                        Alu.is_equal)
                nc.vector.copy_predicated(
                    out=x[:, :, ADDR_KEY:ADDR_KEY + 48],
                    mask=cond[:, :, 0:48], data=ones48[:, :, 0:48])
                nc.vector.copy_predicated(
                    out=x[:, :, MEM_STORE],
                    mask=cond2[:, r, csl], data=ones48[:, :, 0])
                nc.sync.dma_start(out=out_v[r, :, csl, :], in_=x[:])

            for g in range(NG):
                r, t = divmod(g, CPR // CTILE)
                c0 = t * CTILE
                csl = slice(c0, c0 + CTILE)
                x = mainp.tile([P, CTILE, D], bf16, tag="x")
                if g < NDG:
                    nc.gpsimd.dma_gather(
                        out_ap=x[:],
                        in_ap=tab_d[:],
                        idxs_ap=idx_sb[:, g * (NI // 16):(g + 1) * (NI // 16)],
                        num_idxs=NI,
                        num_idxs_reg=NI,
                        elem_size=D,
                    )
                else:
                    for u in range(2):          # half-group = 4 columns
                        q0 = g * CTILE + u * 4  # global column index base
                        bc = bcp.tile([P, 4 * P], f32, tag="bc")
                        nc.tensor.matmul(bc[:], ones1[:],
                                         tokr_sb[:, q0 * P:(q0 + 4) * P],
                                         start=True, stop=True)
                        oh = ohp.tile([P, 3, 4 * P], bf16, tag="oh")
                        # vocab-chunk one-hots (must be DVE: Pool can't read
                        # PSUM, ACT has no is_equal)
                        nc.vector.tensor_scalar(oh[:, 0, :], bc[:],
                                                iotav[:, 0:1], None, Alu.is_equal)
                        nc.vector.tensor_scalar(oh[:, 1, :], bc[:],
                                                iotav[:, 1:2], None, Alu.is_equal)
                        nc.vector.tensor_scalar(oh[0:16, 2, :], bc[0:16, :],
                                                iotav[0:16, 2:3], None,
                                                Alu.is_equal)
                        for jj in range(4):
                            xp = outp.tile([P, D], f32, tag="xp")
                            msl = slice(jj * P, (jj + 1) * P)
                            for k, tb in enumerate(tabs):
                                kp = tb.shape[0]
                                nc.tensor.matmul(xp[:], oh[0:kp, k, msl], tb[:],
                                                 start=(k == 0), stop=(k == 2))
                            nc.scalar.copy(x[:, u * 4 + jj, :], xp[:])
                patch_and_store(x, r, csl)
    nc.finalize()
    return nc


def _get_nc(mhe: int):
    if mhe not in _CACHE:
        _CACHE[mhe] = _build(mhe)
    return _CACHE[mhe]


def _make_idx(tok_core: np.ndarray) -> np.ndarray:
    """[128, IDXW] int16 idx tensor for the NDG dma_gather groups.

    dma_gather: dst[p, j, :] = table[I[j*128 + p]] with I[i] =
    idxs[i % 16, i // 16] (16-partition wrap, replicated 8x to 128
    partitions).  Group g covers PM columns (r, c): r = g // 8,
    c in [8*(g%8), 8*(g%8)+8); dst[p, j] must be token (r, 64p + c0 + j).
    """
    A = tok_core.reshape(RPC, P, CPR)          # A[r, p, c] = tok[r, 64p + c]
    cols = []
    for g in range(NDG):
        r, t = divmod(g, CPR // CTILE)
        blk = A[r, :, t * CTILE:(t + 1) * CTILE]      # [128, CTILE]
        I = np.ascontiguousarray(blk.T).reshape(-1)   # I[j*128 + p]
        W = np.ascontiguousarray(I.reshape(-1, 16).T)  # [16, NI/16]
        cols.append(np.tile(W, (8, 1)))               # [128, NI/16]
    return np.ascontiguousarray(np.concatenate(cols, axis=1).astype(np.int16))


def _make_tokr(tok_core: np.ndarray) -> np.ndarray:
    """[1, NTOK] f16 token row for the PE groups: tokr[q*128 + p] =
    tok[r, 64p + c] with q = r*64 + c (fp16 is exact for vocab < 2048)."""
    A = tok_core.reshape(RPC, P, CPR)
    return np.ascontiguousarray(
        A.transpose(0, 2, 1).reshape(1, NTOK).astype(np.float16))


def _in_maps(token_ids, embed_table):
    import ml_dtypes
    tok = np.asarray(token_ids)
    tab = np.asarray(embed_table, dtype=np.float32)
    assert tok.shape == (B, S) and tab.shape == (V, D)
    tok = np.ascontiguousarray(tok.astype(np.int32, copy=False))
    tab16 = np.ascontiguousarray(tab.astype(ml_dtypes.bfloat16))
    maps = []
    for c in range(NCORES):
        tok_core = tok[c * RPC:(c + 1) * RPC]
        maps.append({
            "tok": tok_core,
            "idx": _make_idx(tok_core),
            "tokr": _make_tokr(tok_core),
            "table": tab16,
        })
    return maps


def kernel(token_ids, embed_table, mem_history_end):
    from concourse.bass_utils import run_bass_kernel_spmd

    mhe = int(mem_history_end)
    nc = _get_nc(mhe)
    in_maps = _in_maps(token_ids, embed_table)
    res = run_bass_kernel_spmd(nc, in_maps, list(range(NCORES))).results
    out = np.concatenate([np.asarray(res[c]["out"]) for c in range(NCORES)],
                         axis=0)
    return out.reshape(B, S, D).astype(np.float32)


# revision 14
# speedup vs baseline: 1.0104x; 1.0104x over previous
"""Trainium2 Bass kernel for NeuralVMEmbedding (embedding lookup + VM channel injection).

Strategy (pure data-parallel over batch; 8 cores, 4 rows of 8192 tokens each):
  - Table uploaded as bf16 (tolerance 2e-2 >> bf16's ~4e-3), output written
    bf16 and upcast to f32 on host -> halves all HBM/DMA traffic vs f32.
  - Hybrid gather, split per 1024-token group (32 groups/core):
      * NDG groups via gpsimd dma_gather (SWDGE ucode, ~8.8ns/row on Pool;
        >1024 idxs per instruction crashes the SWDGE ring - keep 1024).
        Host-permuted idx list lands tiles directly in PM layout.
      * remaining groups via PE one-hot matmul: token row broadcast with a
        K=1 outer product, one-hot built by is_equal vs per-partition iota,
        3 vocab-chunk matmuls (K=128/128/16) accumulate table rows in PSUM,
        scalar engine converts PSUM f32 -> SBUF bf16.
  - ADDR_KEY one-hot + MEM_STORE injection computed on-chip (scans on DVE,
    copy_predicated patches), identical for both gather paths.
  - Output written back with 8KB-contiguous DMA runs via the sync HWDGE queue.
"""

import sys
import numpy as np

for _p in ("/opt/trn_rl_repo",):
    if _p not in sys.path:
        sys.path.insert(0, _p)

# ---- problem constants (hardcoded per contract) ----
B, S, D, V = 32, 8192, 512, 272
NCORES = 8
RPC = B // NCORES          # rows (batch) per core = 4
P = 128                    # partitions
CPR = S // P               # columns per row in partition-major layout = 64
CTILE = 8                  # tile width in columns (CTILE*128 = 1024 tokens)
NTOK = RPC * S             # tokens per core = 32768
NG = NTOK // (P * CTILE)   # groups per core = 32
NDG = 12                   # groups gathered via dma_gather; rest via PE
IDXW = NDG * P * CTILE // 16
ADDR_KEY = 206
MEM_STORE = 455

_CACHE = {}


def _build(mhe: int):
    from concourse import bass, bacc, mybir, tile

    f32 = mybir.dt.float32
    f16 = mybir.dt.float16
    bf16 = mybir.dt.bfloat16
    i32 = mybir.dt.int32
    i16 = mybir.dt.int16
    u8 = mybir.dt.uint8
    Alu = mybir.AluOpType

    nc = bacc.Bacc(None)
    tok_d = nc.declare_dram_parameter("tok", [RPC, S], i32, isOutput=False)
    idx_d = nc.declare_dram_parameter("idx", [P, IDXW], i16, isOutput=False)
    tokr_d = nc.declare_dram_parameter("tokr", [1, NTOK], f16, isOutput=False)
    tab_d = nc.declare_dram_parameter("table", [V, D], bf16, isOutput=False)
    out_d = nc.declare_dram_parameter("out", [RPC, S, D], bf16, isOutput=True)

    with tile.TileContext(nc) as tc:
        with tc.tile_pool(name="const", bufs=1) as constp, \
             tc.tile_pool(name="pre", bufs=1) as pre, \
             tc.tile_pool(name="dramp", bufs=1, space="DRAM") as dramp, \
             tc.tile_pool(name="mainp", bufs=4) as mainp, \
             tc.tile_pool(name="condp", bufs=3) as condp, \
             tc.tile_pool(name="ohp", bufs=3) as ohp, \
             tc.tile_pool(name="bcp", bufs=2, space="PSUM") as bcp, \
             tc.tile_pool(name="outp", bufs=3, space="PSUM") as outp:

            # ---------------- constants ----------------
            iota48_i = constp.tile([P, CTILE, 3, 16], i32)
            nc.gpsimd.iota(iota48_i[:], pattern=[[0, CTILE], [0, 3], [1, 16]],
                           base=0, channel_multiplier=0)
            iota48 = constp.tile([P, CTILE, 3, 16], f32)
            nc.vector.tensor_copy(iota48[:], iota48_i[:])

            # padded to 64 in the last dim so [:, :, 0:48] slices keep a
            # 3-D access pattern matching the strided x[...] views
            ones48 = constp.tile([P, CTILE, 64], bf16)
            nc.vector.memset(ones48[:], 1.0)

            pos_i = constp.tile([P, RPC, CPR], i32)   # pos = 64*p + c (per row)
            nc.gpsimd.iota(pos_i[:], pattern=[[0, RPC], [1, CPR]], base=0,
                           channel_multiplier=CPR)
            pos_f = constp.tile([P, RPC, CPR], f32)
            nc.vector.tensor_copy(pos_f[:], pos_i[:])

            # PE-gather constants: table chunks, ones row, per-chunk iota cols
            tab1 = constp.tile([P, D], bf16)
            nc.sync.dma_start(out=tab1[:], in_=tab_d[0:128, :])
            tab2 = constp.tile([P, D], bf16)
            nc.sync.dma_start(out=tab2[:], in_=tab_d[128:256, :])
            tab3 = constp.tile([16, D], bf16)
            nc.sync.dma_start(out=tab3[:], in_=tab_d[256:272, :])
            tabs = (tab1, tab2, tab3)

            ones1 = constp.tile([1, P], f16)
            nc.vector.memset(ones1[:], 1.0)

            iotav_i = constp.tile([P, 3], i32)        # p + 128k
            nc.gpsimd.iota(iotav_i[:], pattern=[[128, 3]], base=0,
                           channel_multiplier=1)
            iotav = constp.tile([P, 3], f32)
            nc.vector.tensor_copy(iotav[:], iotav_i[:])

            tokr_sb = constp.tile([1, NTOK], f16)
            nc.sync.dma_start(out=tokr_sb[:], in_=tokr_d[:])

            # ---------------- token / idx load ----------------
            tok_i = pre.tile([P, RPC, CPR], i32)
            nc.sync.dma_start(out=tok_i[:],
                              in_=tok_d[:].rearrange("r (p c) -> p r c", p=P))
            tok_f = pre.tile([P, RPC, CPR], f32)
            nc.vector.tensor_copy(tok_f[:], tok_i[:])

            idx_sb = pre.tile([P, IDXW], i16)
            nc.sync.dma_start(out=idx_sb[:], in_=idx_d[:])

            # ---------------- scan inputs ----------------
            posp1 = pre.tile([P, RPC, CPR], f32)
            nc.vector.tensor_scalar(posp1[:], pos_f[:], 1.0, None, Alu.add)
            posm1 = pre.tile([P, RPC, CPR], f32)
            nc.vector.tensor_scalar(posm1[:], pos_f[:], 1.0, None, Alu.subtract)

            # v0 = (tok==256)*(pos+1) - 1   (CODE_START candidate positions)
            v0 = pre.tile([P, RPC, CPR], f32)
            nc.vector.scalar_tensor_tensor(v0[:], tok_f[:], 256.0, posp1[:],
                                           Alu.is_equal, Alu.mult)
            nc.vector.tensor_scalar(v0[:], v0[:], 1.0, None, Alu.subtract)

            # v1 = (tok==257)  (CODE_END seen)
            v1 = pre.tile([P, RPC, CPR], f32)
            nc.vector.tensor_scalar(v1[:], tok_f[:], 257.0, None, Alu.is_equal)

            cs = pre.tile([P, RPC, CPR], f32)
            ce = pre.tile([P, RPC, CPR], f32)

            # --- level 1: within-partition prefix max over 64-token chunks ---
            loc_cs = pre.tile([P, RPC, CPR], f32)
            loc_ce = pre.tile([P, RPC, CPR], f32)
            for r in range(RPC):
                nc.vector.tensor_tensor_scan(loc_cs[:, r, :], v0[:, r, :],
                                             v0[:, r, :], -1.0,
                                             Alu.max, Alu.bypass)
                nc.vector.tensor_tensor_scan(loc_ce[:, r, :], v1[:, r, :],
                                             v1[:, r, :], 0.0,
                                             Alu.max, Alu.bypass)

            # --- level 2: exclusive prefix max across partitions (chunks) ---
            # Collect the 8 per-partition chunk-final columns (cs rows 0-3,
            # ce rows 4-7), transpose [128, 8] -> [8, 128] via a tiny DRAM
            # round-trip, scan along the free dim, shift for exclusivity,
            # transpose back.
            NS = 2 * RPC
            f8 = pre.tile([P, NS], f32)
            for r in range(RPC):
                nc.vector.tensor_copy(f8[:, r:r + 1],
                                      loc_cs[:, r, CPR - 1:CPR])
                nc.vector.tensor_copy(f8[:, RPC + r:RPC + r + 1],
                                      loc_ce[:, r, CPR - 1:CPR])
            f8_d = dramp.tile([P, NS], f32)
            nc.sync.dma_start(out=f8_d[:], in_=f8[:])
            f8t = pre.tile([NS, P], f32)
            nc.sync.dma_start(out=f8t[:], in_=f8_d[:].rearrange("p j -> j p"))
            p8 = pre.tile([NS, P], f32)
            nc.vector.tensor_tensor_scan(p8[:], f8t[:], f8t[:], -1e30,
                                         Alu.max, Alu.bypass)
            e8t = pre.tile([NS, P], f32)
            # -1 is a neutral carry for both scans (cs values >= -1, ce >= 0)
            nc.vector.memset(e8t[:, 0:1], -1.0)
            nc.vector.tensor_copy(e8t[:, 1:P], p8[:, 0:P - 1])
            e8_d = dramp.tile([NS, P], f32)
            nc.sync.dma_start(out=e8_d[:], in_=e8t[:])
            e8 = pre.tile([P, NS], f32)
            nc.sync.dma_start(out=e8[:], in_=e8_d[:].rearrange("j p -> p j"))

            # --- combine ---
            for r in range(RPC):
                nc.vector.tensor_scalar(cs[:, r, :], loc_cs[:, r, :],
                                        e8[:, r:r + 1], None, Alu.max)
                nc.vector.tensor_scalar(ce[:, r, :], loc_ce[:, r, :],
                                        e8[:, RPC + r:RPC + r + 1], None,
                                        Alu.max)

            # ---------------- per-token derived values ----------------
            # mask = (cs >= 0) & (ce == 0) & (tok < 256)
            m3 = pre.tile([P, RPC, CPR], f32)
            nc.vector.tensor_scalar(m3[:], tok_f[:], 255.5, None, Alu.is_lt)
            m23 = pre.tile([P, RPC, CPR], f32)
            nc.vector.scalar_tensor_tensor(m23[:], ce[:], 0.5, m3[:],
                                           Alu.is_lt, Alu.mult)
            mask = pre.tile([P, RPC, CPR], f32)
            nc.vector.scalar_tensor_tensor(mask[:], cs[:], 0.0, m23[:],
                                           Alu.is_ge, Alu.mult)

            # seq_pos = max(pos - 1 - cs, 0)
            sp = pre.tile([P, RPC, CPR], f32)
            nc.vector.scalar_tensor_tensor(sp[:], cs[:], -1.0, posm1[:],
                                           Alu.mult, Alu.add)
            nc.vector.tensor_scalar(sp[:], sp[:], 0.0, None, Alu.max)

            # q = floor(sp / 5), robust to cast rounding mode:
            #   y = sp*0.2 ; q0 = int(y) ; q = q0 - (y - float(q0) < 0)
            y = pre.tile([P, RPC, CPR], f32)
            nc.vector.tensor_scalar(y[:], sp[:], 0.2, None, Alu.mult)
            q_i = pre.tile([P, RPC, CPR], i32)
            nc.vector.tensor_copy(q_i[:], y[:])
            q_f = pre.tile([P, RPC, CPR], f32)
            nc.vector.tensor_copy(q_f[:], q_i[:])
            corr = pre.tile([P, RPC, CPR], f32)
            nc.vector.tensor_tensor(corr[:], y[:], q_f[:], Alu.subtract)
            nc.vector.tensor_scalar(corr[:], corr[:], 0.0, None, Alu.is_lt)
            nc.vector.tensor_tensor(q_f[:], q_f[:], corr[:], Alu.subtract)

            # addr = sp + 3*q  (int32)
            sp_i = pre.tile([P, RPC, CPR], i32)
            nc.vector.tensor_copy(sp_i[:], sp[:])
            q_i2 = pre.tile([P, RPC, CPR], i32)
            nc.vector.tensor_copy(q_i2[:], q_f[:])
            q3 = pre.tile([P, RPC, CPR], i32)
            nc.vector.tensor_scalar(q3[:], q_i2[:], 1, None, Alu.logical_shift_left)
            nc.vector.tensor_tensor(q3[:], q3[:], q_i2[:], Alu.add)
            addr = pre.tile([P, RPC, CPR], i32)
            nc.vector.tensor_tensor(addr[:], sp_i[:], q3[:], Alu.add)

            # nibbles
            lo_i = pre.tile([P, RPC, CPR], i32)
            nc.vector.tensor_scalar(lo_i[:], addr[:], 15, None, Alu.bitwise_and)
            hi_i = pre.tile([P, RPC, CPR], i32)
            nc.vector.tensor_scalar(hi_i[:], addr[:], 4, 15,
                                    Alu.logical_shift_right, Alu.bitwise_and)
            top_i = pre.tile([P, RPC, CPR], i32)
            nc.vector.tensor_scalar(top_i[:], addr[:], 8, 15,
                                    Alu.logical_shift_right, Alu.bitwise_and)
            # masked nibbles: nib_m = nib + 16*(1-mask) -- unmasked tokens
            # get an out-of-range value (>=16) so the iota 0..15 compare in
            # the cond build never fires; kills the separate mask multiply.
            # (single allocation: same-call-site tiles alias in a bufs=1 pool)
            nm3 = pre.tile([P, 3, RPC, CPR], f32)
            for b, src_i in enumerate((lo_i, hi_i, top_i)):
                nc.vector.tensor_copy(nm3[:, b], src_i[:])
            nc.vector.tensor_scalar(nm3[:], nm3[:], 16.0, None, Alu.add)
            for b in range(3):
                nc.vector.scalar_tensor_tensor(nm3[:, b], mask[:], -16.0,
                                               nm3[:, b], Alu.mult, Alu.add)

            # cond2 = (tok == 258) & (pos < mem_history_end)
            m5 = pre.tile([P, RPC, CPR], f32)
            nc.vector.tensor_scalar(m5[:], pos_f[:], float(mhe), None, Alu.is_lt)
            cond2 = pre.tile([P, RPC, CPR], u8)
            nc.vector.scalar_tensor_tensor(cond2[:], tok_f[:], 258.0, m5[:],
                                           Alu.is_equal, Alu.mult)

            # ---------------- main gather + patch + store loop ----------------
            out_v = out_d[:].rearrange("r (p c) d -> r p c d", p=P)
            NI = P * CTILE                 # tokens per group = 1024

            def patch_and_store(x, r, csl):
                cond = condp.tile([P, CTILE, 64], u8, tag="cond")
                for b in range(3):
                    nc.vector.tensor_tensor(
                        cond[:, :, 16 * b:16 * (b + 1)],
                        iota48[:, :, b, :],
                        nm3[:, b, r, csl].to_broadcast([P, CTILE, 16]),
                        Alu.is_equal)
                nc.vector.copy_predicated(
                    out=x[:, :, ADDR_KEY:ADDR_KEY + 48],
                    mask=cond[:, :, 0:48], data=ones48[:, :, 0:48])
                nc.vector.copy_predicated(
                    out=x[:, :, MEM_STORE],
                    mask=cond2[:, r, csl], data=ones48[:, :, 0])
                nc.sync.dma_start(out=out_v[r, :, csl, :], in_=x[:])

            # interleave dma_gather groups among PE groups so the Pool
            # gathers overlap PE work instead of serializing ahead of it
            dma_groups = list(range(NDG))
            pe_groups = list(range(NDG, NG))
            order = []
            di, pi = 0.0, 0
            ratio = len(pe_groups) / NDG
            for g in range(NG):
                if pi < len(pe_groups) and (di >= NDG or (pi + 1) / (di + 1) <= ratio):
                    order.append(pe_groups[pi]); pi += 1
                else:
                    order.append(dma_groups[int(di)]); di += 1

            for g in order:
                r, t = divmod(g, CPR // CTILE)
                c0 = t * CTILE
                csl = slice(c0, c0 + CTILE)
                x = mainp.tile([P, CTILE, D], bf16, tag="x")
                if g < NDG:
                    nc.gpsimd.dma_gather(
                        out_ap=x[:],
                        in_ap=tab_d[:],
                        idxs_ap=idx_sb[:, g * (NI // 16):(g + 1) * (NI // 16)],
                        num_idxs=NI,
                        num_idxs_reg=NI,
                        elem_size=D,
                    )
                else:
                    for u in range(2):          # half-group = 4 columns
                        q0 = g * CTILE + u * 4  # global column index base
                        bc = bcp.tile([P, 4 * P], f32, tag="bc")
                        nc.tensor.matmul(bc[:], ones1[:],
                                         tokr_sb[:, q0 * P:(q0 + 4) * P],
                                         start=True, stop=True)
                        oh = ohp.tile([P, 3, 4 * P], bf16, tag="oh")
                        # vocab-chunk one-hots (must be DVE: Pool can't read
                        # PSUM, ACT has no is_equal)
                        nc.vector.tensor_scalar(oh[:, 0, :], bc[:],
                                                iotav[:, 0:1], None, Alu.is_equal)
                        nc.vector.tensor_scalar(oh[:, 1, :], bc[:],
                                                iotav[:, 1:2], None, Alu.is_equal)
                        nc.vector.tensor_scalar(oh[0:16, 2, :], bc[0:16, :],
                                                iotav[0:16, 2:3], None,
                                                Alu.is_equal)
                        for v in range(2):      # 2 columns share one PSUM pair
                            xp = outp.tile([P, 2, D], f32, tag="xp")
                            for w in range(2):
                                jj = 2 * v + w
                                msl = slice(jj * P, (jj + 1) * P)
                                for k, tb in enumerate(tabs):
                                    kp = tb.shape[0]
                                    nc.tensor.matmul(xp[:, w, :],
                                                     oh[0:kp, k, msl], tb[:],
                                                     start=(k == 0),
                                                     stop=(k == 2))
                            nc.scalar.copy(
                                x[:, u * 4 + 2 * v:u * 4 + 2 * v + 2, :],
                                xp[:])
                patch_and_store(x, r, csl)
    nc.finalize()
    return nc


def _get_nc(mhe: int):
    if mhe not in _CACHE:
        _CACHE[mhe] = _build(mhe)
    return _CACHE[mhe]


def _make_idx(tok_core: np.ndarray) -> np.ndarray:
    """[128, IDXW] int16 idx tensor for the NDG dma_gather groups.

    dma_gather: dst[p, j, :] = table[I[j*128 + p]] with I[i] =
    idxs[i % 16, i // 16] (16-partition wrap, replicated 8x to 128
    partitions).  Group g covers PM columns (r, c): r = g // 8,
    c in [8*(g%8), 8*(g%8)+8); dst[p, j] must be token (r, 64p + c0 + j).
    """
    A = tok_core.reshape(RPC, P, CPR)          # A[r, p, c] = tok[r, 64p + c]
    cols = []
    for g in range(NDG):
        r, t = divmod(g, CPR // CTILE)
        blk = A[r, :, t * CTILE:(t + 1) * CTILE]      # [128, CTILE]
        I = np.ascontiguousarray(blk.T).reshape(-1)   # I[j*128 + p]
        W = np.ascontiguousarray(I.reshape(-1, 16).T)  # [16, NI/16]
        cols.append(np.tile(W, (8, 1)))               # [128, NI/16]
    return np.ascontiguousarray(np.concatenate(cols, axis=1).astype(np.int16))


def _make_tokr(tok_core: np.ndarray) -> np.ndarray:
    """[1, NTOK] f16 token row for the PE groups: tokr[q*128 + p] =
    tok[r, 64p + c] with q = r*64 + c (fp16 is exact for vocab < 2048)."""
    A = tok_core.reshape(RPC, P, CPR)
    return np.ascontiguousarray(
        A.transpose(0, 2, 1).reshape(1, NTOK).astype(np.float16))


def _in_maps(token_ids, embed_table):
    import ml_dtypes
    tok = np.asarray(token_ids)
    tab = np.asarray(embed_table, dtype=np.float32)
    assert tok.shape == (B, S) and tab.shape == (V, D)
    tok = np.ascontiguousarray(tok.astype(np.int32, copy=False))
    tab16 = np.ascontiguousarray(tab.astype(ml_dtypes.bfloat16))
    maps = []
    for c in range(NCORES):
        tok_core = tok[c * RPC:(c + 1) * RPC]
        maps.append({
            "tok": tok_core,
            "idx": _make_idx(tok_core),
            "tokr": _make_tokr(tok_core),
            "table": tab16,
        })
    return maps


def kernel(token_ids, embed_table, mem_history_end):
    from concourse.bass_utils import run_bass_kernel_spmd

    mhe = int(mem_history_end)
    nc = _get_nc(mhe)
    in_maps = _in_maps(token_ids, embed_table)
    res = run_bass_kernel_spmd(nc, in_maps, list(range(NCORES))).results
    out = np.concatenate([np.asarray(res[c]["out"]) for c in range(NCORES)],
                         axis=0)
    return out.reshape(B, S, D).astype(np.float32)


# revision 15
# speedup vs baseline: 1.3281x; 1.3144x over previous
"""Trainium2 Bass kernel for NeuralVMEmbedding (embedding lookup + VM channel injection).

Strategy (pure data-parallel over batch; 8 cores, 4 rows of 8192 tokens each):
  - Table uploaded as bf16 (tolerance 2e-2 >> bf16's ~4e-3), output written
    bf16 and upcast to f32 on host -> halves all HBM/DMA traffic vs f32.
  - Hybrid gather, split per 1024-token group (32 groups/core):
      * NDG groups via gpsimd dma_gather (SWDGE ucode, ~8.8ns/row on Pool;
        >1024 idxs per instruction crashes the SWDGE ring - keep 1024).
        Host-permuted idx list lands tiles directly in PM layout.
      * remaining groups via PE one-hot matmul: token row broadcast with a
        K=1 outer product, one-hot built by is_equal vs per-partition iota,
        3 vocab-chunk matmuls (K=128/128/16) accumulate table rows in PSUM,
        scalar engine converts PSUM f32 -> SBUF bf16.
  - ADDR_KEY one-hot + MEM_STORE injection computed on-chip (scans on DVE,
    copy_predicated patches), identical for both gather paths.
  - Output written back with 8KB-contiguous DMA runs via the sync HWDGE queue.
"""

import sys
import numpy as np

for _p in ("/opt/trn_rl_repo",):
    if _p not in sys.path:
        sys.path.insert(0, _p)

# ---- problem constants (hardcoded per contract) ----
B, S, D, V = 32, 8192, 512, 272
NCORES = 8
RPC = B // NCORES          # rows (batch) per core = 4
P = 128                    # partitions
CPR = S // P               # columns per row in partition-major layout = 64
CTILE = 8                  # tile width in columns (CTILE*128 = 1024 tokens)
NTOK = RPC * S             # tokens per core = 32768
NG = NTOK // (P * CTILE)   # groups per core = 32
NDG = 21                   # groups gathered via dma_gather; rest via PE
IDXW = NDG * P * CTILE // 16
ADDR_KEY = 206
MEM_STORE = 455

_CACHE = {}


def _build(mhe: int):
    from concourse import bass, bacc, mybir, tile

    f32 = mybir.dt.float32
    f16 = mybir.dt.float16
    bf16 = mybir.dt.bfloat16
    i32 = mybir.dt.int32
    i16 = mybir.dt.int16
    u8 = mybir.dt.uint8
    Alu = mybir.AluOpType

    nc = bacc.Bacc(None)
    tok_d = nc.declare_dram_parameter("tok", [RPC, S], i32, isOutput=False)
    idx_d = nc.declare_dram_parameter("idx", [P, IDXW], i16, isOutput=False)
    tokr_d = nc.declare_dram_parameter("tokr", [1, NTOK], f16, isOutput=False)
    tab_d = nc.declare_dram_parameter("table", [V, D], bf16, isOutput=False)
    out_d = nc.declare_dram_parameter("out", [RPC, S, D], bf16, isOutput=True)

    with tile.TileContext(nc) as tc:
        with tc.tile_pool(name="const", bufs=1) as constp, \
             tc.tile_pool(name="pre", bufs=1) as pre, \
             tc.tile_pool(name="dramp", bufs=1, space="DRAM") as dramp, \
             tc.tile_pool(name="mainp", bufs=4) as mainp, \
             tc.tile_pool(name="condp", bufs=3) as condp, \
             tc.tile_pool(name="ohp", bufs=3) as ohp, \
             tc.tile_pool(name="bcp", bufs=2, space="PSUM") as bcp, \
             tc.tile_pool(name="outp", bufs=3, space="PSUM") as outp:

            # ---------------- constants ----------------
            iota48_i = constp.tile([P, CTILE, 3, 16], i32)
            nc.gpsimd.iota(iota48_i[:], pattern=[[0, CTILE], [0, 3], [1, 16]],
                           base=0, channel_multiplier=0)
            iota48 = constp.tile([P, CTILE, 3, 16], f32)
            nc.vector.tensor_copy(iota48[:], iota48_i[:])

            # padded to 64 in the last dim so [:, :, 0:48] slices keep a
            # 3-D access pattern matching the strided x[...] views
            ones48 = constp.tile([P, CTILE, 64], bf16)
            nc.vector.memset(ones48[:], 1.0)

            pos_i = constp.tile([P, RPC, CPR], i32)   # pos = 64*p + c (per row)
            nc.gpsimd.iota(pos_i[:], pattern=[[0, RPC], [1, CPR]], base=0,
                           channel_multiplier=CPR)
            pos_f = constp.tile([P, RPC, CPR], f32)
            nc.vector.tensor_copy(pos_f[:], pos_i[:])

            # PE-gather constants: table chunks, ones row, per-chunk iota cols
            tab1 = constp.tile([P, D], bf16)
            nc.sync.dma_start(out=tab1[:], in_=tab_d[0:128, :])
            tab2 = constp.tile([P, D], bf16)
            nc.sync.dma_start(out=tab2[:], in_=tab_d[128:256, :])
            tab3 = constp.tile([16, D], bf16)
            nc.sync.dma_start(out=tab3[:], in_=tab_d[256:272, :])
            tabs = (tab1, tab2, tab3)

            ones1 = constp.tile([1, P], f16)
            nc.vector.memset(ones1[:], 1.0)

            iotav_i = constp.tile([P, 3], i32)        # p + 128k
            nc.gpsimd.iota(iotav_i[:], pattern=[[128, 3]], base=0,
                           channel_multiplier=1)
            iotav = constp.tile([P, 3], f32)
            nc.vector.tensor_copy(iotav[:], iotav_i[:])

            tokr_sb = constp.tile([1, NTOK], f16)
            nc.sync.dma_start(out=tokr_sb[:], in_=tokr_d[:])

            # ---------------- token / idx load ----------------
            tok_i = pre.tile([P, RPC, CPR], i32)
            nc.sync.dma_start(out=tok_i[:],
                              in_=tok_d[:].rearrange("r (p c) -> p r c", p=P))
            tok_f = pre.tile([P, RPC, CPR], f32)
            nc.vector.tensor_copy(tok_f[:], tok_i[:])

            idx_sb = pre.tile([P, IDXW], i16)
            nc.sync.dma_start(out=idx_sb[:], in_=idx_d[:])

            # ---------------- scan inputs ----------------
            posp1 = pre.tile([P, RPC, CPR], f32)
            nc.vector.tensor_scalar(posp1[:], pos_f[:], 1.0, None, Alu.add)
            posm1 = pre.tile([P, RPC, CPR], f32)
            nc.vector.tensor_scalar(posm1[:], pos_f[:], 1.0, None, Alu.subtract)

            # v0 = (tok==256)*(pos+1) - 1   (CODE_START candidate positions)
            v0 = pre.tile([P, RPC, CPR], f32)
            nc.vector.scalar_tensor_tensor(v0[:], tok_f[:], 256.0, posp1[:],
                                           Alu.is_equal, Alu.mult)
            nc.vector.tensor_scalar(v0[:], v0[:], 1.0, None, Alu.subtract)

            # v1 = (tok==257)  (CODE_END seen)
            v1 = pre.tile([P, RPC, CPR], f32)
            nc.vector.tensor_scalar(v1[:], tok_f[:], 257.0, None, Alu.is_equal)

            cs = pre.tile([P, RPC, CPR], f32)
            ce = pre.tile([P, RPC, CPR], f32)

            # --- level 1: within-partition prefix max over 64-token chunks ---
            loc_cs = pre.tile([P, RPC, CPR], f32)
            loc_ce = pre.tile([P, RPC, CPR], f32)
            for r in range(RPC):
                nc.vector.tensor_tensor_scan(loc_cs[:, r, :], v0[:, r, :],
                                             v0[:, r, :], -1.0,
                                             Alu.max, Alu.bypass)
                nc.vector.tensor_tensor_scan(loc_ce[:, r, :], v1[:, r, :],
                                             v1[:, r, :], 0.0,
                                             Alu.max, Alu.bypass)

            # --- level 2: exclusive prefix max across partitions (chunks) ---
            # Collect the 8 per-partition chunk-final columns (cs rows 0-3,
            # ce rows 4-7), transpose [128, 8] -> [8, 128] via a tiny DRAM
            # round-trip, scan along the free dim, shift for exclusivity,
            # transpose back.
            NS = 2 * RPC
            f8 = pre.tile([P, NS], f32)
            for r in range(RPC):
                nc.vector.tensor_copy(f8[:, r:r + 1],
                                      loc_cs[:, r, CPR - 1:CPR])
                nc.vector.tensor_copy(f8[:, RPC + r:RPC + r + 1],
                                      loc_ce[:, r, CPR - 1:CPR])
            f8_d = dramp.tile([P, NS], f32)
            nc.sync.dma_start(out=f8_d[:], in_=f8[:])
            f8t = pre.tile([NS, P], f32)
            nc.sync.dma_start(out=f8t[:], in_=f8_d[:].rearrange("p j -> j p"))
            p8 = pre.tile([NS, P], f32)
            nc.vector.tensor_tensor_scan(p8[:], f8t[:], f8t[:], -1e30,
                                         Alu.max, Alu.bypass)
            e8t = pre.tile([NS, P], f32)
            # -1 is a neutral carry for both scans (cs values >= -1, ce >= 0)
            nc.vector.memset(e8t[:, 0:1], -1.0)
            nc.vector.tensor_copy(e8t[:, 1:P], p8[:, 0:P - 1])
            e8_d = dramp.tile([NS, P], f32)
            nc.sync.dma_start(out=e8_d[:], in_=e8t[:])
            e8 = pre.tile([P, NS], f32)
            nc.sync.dma_start(out=e8[:], in_=e8_d[:].rearrange("j p -> p j"))

            # --- combine ---
            for r in range(RPC):
                nc.vector.tensor_scalar(cs[:, r, :], loc_cs[:, r, :],
                                        e8[:, r:r + 1], None, Alu.max)
                nc.vector.tensor_scalar(ce[:, r, :], loc_ce[:, r, :],
                                        e8[:, RPC + r:RPC + r + 1], None,
                                        Alu.max)

            # ---------------- per-token derived values ----------------
            # mask = (cs >= 0) & (ce == 0) & (tok < 256)
            m3 = pre.tile([P, RPC, CPR], f32)
            nc.vector.tensor_scalar(m3[:], tok_f[:], 255.5, None, Alu.is_lt)
            m23 = pre.tile([P, RPC, CPR], f32)
            nc.vector.scalar_tensor_tensor(m23[:], ce[:], 0.5, m3[:],
                                           Alu.is_lt, Alu.mult)
            mask = pre.tile([P, RPC, CPR], f32)
            nc.vector.scalar_tensor_tensor(mask[:], cs[:], 0.0, m23[:],
                                           Alu.is_ge, Alu.mult)

            # seq_pos = max(pos - 1 - cs, 0)
            sp = pre.tile([P, RPC, CPR], f32)
            nc.vector.scalar_tensor_tensor(sp[:], cs[:], -1.0, posm1[:],
                                           Alu.mult, Alu.add)
            nc.vector.tensor_scalar(sp[:], sp[:], 0.0, None, Alu.max)

            # q = floor(sp / 5), robust to cast rounding mode:
            #   y = sp*0.2 ; q0 = int(y) ; q = q0 - (y - float(q0) < 0)
            y = pre.tile([P, RPC, CPR], f32)
            nc.vector.tensor_scalar(y[:], sp[:], 0.2, None, Alu.mult)
            q_i = pre.tile([P, RPC, CPR], i32)
            nc.vector.tensor_copy(q_i[:], y[:])
            q_f = pre.tile([P, RPC, CPR], f32)
            nc.vector.tensor_copy(q_f[:], q_i[:])
            corr = pre.tile([P, RPC, CPR], f32)
            nc.vector.tensor_tensor(corr[:], y[:], q_f[:], Alu.subtract)
            nc.vector.tensor_scalar(corr[:], corr[:], 0.0, None, Alu.is_lt)
            nc.vector.tensor_tensor(q_f[:], q_f[:], corr[:], Alu.subtract)

            # addr = sp + 3*q  (int32)
            sp_i = pre.tile([P, RPC, CPR], i32)
            nc.vector.tensor_copy(sp_i[:], sp[:])
            q_i2 = pre.tile([P, RPC, CPR], i32)
            nc.vector.tensor_copy(q_i2[:], q_f[:])
            q3 = pre.tile([P, RPC, CPR], i32)
            nc.vector.tensor_scalar(q3[:], q_i2[:], 1, None, Alu.logical_shift_left)
            nc.vector.tensor_tensor(q3[:], q3[:], q_i2[:], Alu.add)
            addr = pre.tile([P, RPC, CPR], i32)
            nc.vector.tensor_tensor(addr[:], sp_i[:], q3[:], Alu.add)

            # nibbles
            lo_i = pre.tile([P, RPC, CPR], i32)
            nc.vector.tensor_scalar(lo_i[:], addr[:], 15, None, Alu.bitwise_and)
            hi_i = pre.tile([P, RPC, CPR], i32)
            nc.vector.tensor_scalar(hi_i[:], addr[:], 4, 15,
                                    Alu.logical_shift_right, Alu.bitwise_and)
            top_i = pre.tile([P, RPC, CPR], i32)
            nc.vector.tensor_scalar(top_i[:], addr[:], 8, 15,
                                    Alu.logical_shift_right, Alu.bitwise_and)
            # masked nibbles: nib_m = nib + 16*(1-mask) -- unmasked tokens
            # get an out-of-range value (>=16) so the iota 0..15 compare in
            # the cond build never fires; kills the separate mask multiply.
            # (single allocation: same-call-site tiles alias in a bufs=1 pool)
            nm3 = pre.tile([P, 3, RPC, CPR], f32)
            for b, src_i in enumerate((lo_i, hi_i, top_i)):
                nc.vector.tensor_copy(nm3[:, b], src_i[:])
            nc.vector.tensor_scalar(nm3[:], nm3[:], 16.0, None, Alu.add)
            for b in range(3):
                nc.vector.scalar_tensor_tensor(nm3[:, b], mask[:], -16.0,
                                               nm3[:, b], Alu.mult, Alu.add)

            # cond2 = (tok == 258) & (pos < mem_history_end)
            m5 = pre.tile([P, RPC, CPR], f32)
            nc.vector.tensor_scalar(m5[:], pos_f[:], float(mhe), None, Alu.is_lt)
            cond2 = pre.tile([P, RPC, CPR], u8)
            nc.vector.scalar_tensor_tensor(cond2[:], tok_f[:], 258.0, m5[:],
                                           Alu.is_equal, Alu.mult)

            # ---------------- main gather + patch + store loop ----------------
            out_v = out_d[:].rearrange("r (p c) d -> r p c d", p=P)
            NI = P * CTILE                 # tokens per group = 1024

            def patch_and_store(x, r, csl):
                cond = condp.tile([P, CTILE, 64], u8, tag="cond")
                for b in range(3):
                    nc.vector.tensor_tensor(
                        cond[:, :, 16 * b:16 * (b + 1)],
                        iota48[:, :, b, :],
                        nm3[:, b, r, csl].to_broadcast([P, CTILE, 16]),
                        Alu.is_equal)
                nc.vector.copy_predicated(
                    out=x[:, :, ADDR_KEY:ADDR_KEY + 48],
                    mask=cond[:, :, 0:48], data=ones48[:, :, 0:48])
                nc.vector.copy_predicated(
                    out=x[:, :, MEM_STORE],
                    mask=cond2[:, r, csl], data=ones48[:, :, 0])
                nc.sync.dma_start(out=out_v[r, :, csl, :], in_=x[:])

            # interleave dma_gather groups among PE groups so the Pool
            # gathers overlap PE work instead of serializing ahead of it
            dma_groups = list(range(NDG))
            pe_groups = list(range(NDG, NG))
            order = []
            di, pi = 0.0, 0
            ratio = len(pe_groups) / NDG
            for g in range(NG):
                if pi < len(pe_groups) and (di >= NDG or (pi + 1) / (di + 1) <= ratio):
                    order.append(pe_groups[pi]); pi += 1
                else:
                    order.append(dma_groups[int(di)]); di += 1

            for g in order:
                r, t = divmod(g, CPR // CTILE)
                c0 = t * CTILE
                csl = slice(c0, c0 + CTILE)
                x = mainp.tile([P, CTILE, D], bf16, tag="x")
                if g < NDG:
                    nc.gpsimd.dma_gather(
                        out_ap=x[:],
                        in_ap=tab_d[:],
                        idxs_ap=idx_sb[:, g * (NI // 16):(g + 1) * (NI // 16)],
                        num_idxs=NI,
                        num_idxs_reg=NI,
                        elem_size=D,
                    )
                else:
                    for u in range(2):          # half-group = 4 columns
                        q0 = g * CTILE + u * 4  # global column index base
                        bc = bcp.tile([P, 4 * P], f32, tag="bc")
                        nc.tensor.matmul(bc[:], ones1[:],
                                         tokr_sb[:, q0 * P:(q0 + 4) * P],
                                         start=True, stop=True)
                        oh = ohp.tile([P, 3, 4 * P], bf16, tag="oh")
                        # vocab-chunk one-hots (must be DVE: Pool can't read
                        # PSUM, ACT has no is_equal)
                        nc.vector.tensor_scalar(oh[:, 0, :], bc[:],
                                                iotav[:, 0:1], None, Alu.is_equal)
                        nc.vector.tensor_scalar(oh[:, 1, :], bc[:],
                                                iotav[:, 1:2], None, Alu.is_equal)
                        nc.vector.tensor_scalar(oh[0:16, 2, :], bc[0:16, :],
                                                iotav[0:16, 2:3], None,
                                                Alu.is_equal)
                        for v in range(2):      # 2 columns share one PSUM pair
                            xp = outp.tile([P, 2, D], f32, tag="xp")
                            for w in range(2):
                                jj = 2 * v + w
                                msl = slice(jj * P, (jj + 1) * P)
                                for k, tb in enumerate(tabs):
                                    kp = tb.shape[0]
                                    nc.tensor.matmul(xp[:, w, :],
                                                     oh[0:kp, k, msl], tb[:],
                                                     start=(k == 0),
                                                     stop=(k == 2))
                            nc.scalar.copy(
                                x[:, u * 4 + 2 * v:u * 4 + 2 * v + 2, :],
                                xp[:])
                patch_and_store(x, r, csl)
    nc.finalize()
    return nc


def _get_nc(mhe: int):
    if mhe not in _CACHE:
        _CACHE[mhe] = _build(mhe)
    return _CACHE[mhe]


def _make_idx(tok_core: np.ndarray) -> np.ndarray:
    """[128, IDXW] int16 idx tensor for the NDG dma_gather groups.

    dma_gather: dst[p, j, :] = table[I[j*128 + p]] with I[i] =
    idxs[i % 16, i // 16] (16-partition wrap, replicated 8x to 128
    partitions).  Group g covers PM columns (r, c): r = g // 8,
    c in [8*(g%8), 8*(g%8)+8); dst[p, j] must be token (r, 64p + c0 + j).
    """
    A = tok_core.reshape(RPC, P, CPR)          # A[r, p, c] = tok[r, 64p + c]
    cols = []
    for g in range(NDG):
        r, t = divmod(g, CPR // CTILE)
        blk = A[r, :, t * CTILE:(t + 1) * CTILE]      # [128, CTILE]
        I = np.ascontiguousarray(blk.T).reshape(-1)   # I[j*128 + p]
        W = np.ascontiguousarray(I.reshape(-1, 16).T)  # [16, NI/16]
        cols.append(np.tile(W, (8, 1)))               # [128, NI/16]
    return np.ascontiguousarray(np.concatenate(cols, axis=1).astype(np.int16))


def _make_tokr(tok_core: np.ndarray) -> np.ndarray:
    """[1, NTOK] f16 token row for the PE groups: tokr[q*128 + p] =
    tok[r, 64p + c] with q = r*64 + c (fp16 is exact for vocab < 2048)."""
    A = tok_core.reshape(RPC, P, CPR)
    return np.ascontiguousarray(
        A.transpose(0, 2, 1).reshape(1, NTOK).astype(np.float16))


def _in_maps(token_ids, embed_table):
    import ml_dtypes
    tok = np.asarray(token_ids)
    tab = np.asarray(embed_table, dtype=np.float32)
    assert tok.shape == (B, S) and tab.shape == (V, D)
    tok = np.ascontiguousarray(tok.astype(np.int32, copy=False))
    tab16 = np.ascontiguousarray(tab.astype(ml_dtypes.bfloat16))
    maps = []
    for c in range(NCORES):
        tok_core = tok[c * RPC:(c + 1) * RPC]
        maps.append({
            "tok": tok_core,
            "idx": _make_idx(tok_core),
            "tokr": _make_tokr(tok_core),
            "table": tab16,
        })
    return maps


def kernel(token_ids, embed_table, mem_history_end):
    from concourse.bass_utils import run_bass_kernel_spmd

    mhe = int(mem_history_end)
    nc = _get_nc(mhe)
    in_maps = _in_maps(token_ids, embed_table)
    res = run_bass_kernel_spmd(nc, in_maps, list(range(NCORES))).results
    out = np.concatenate([np.asarray(res[c]["out"]) for c in range(NCORES)],
                         axis=0)
    return out.reshape(B, S, D).astype(np.float32)


# revision 16
# speedup vs baseline: 1.4444x; 1.0876x over previous
"""Trainium2 Bass kernel for NeuralVMEmbedding (embedding lookup + VM channel injection).

Strategy (pure data-parallel over batch; 8 cores, 4 rows of 8192 tokens each):
  - Table uploaded as bf16 (tolerance 2e-2 >> bf16's ~4e-3), output written
    bf16 and upcast to f32 on host -> halves all HBM/DMA traffic vs f32.
  - Hybrid gather, split per 1024-token group (32 groups/core):
      * NDG groups via gpsimd dma_gather (SWDGE ucode, ~8.8ns/row on Pool;
        >1024 idxs per instruction crashes the SWDGE ring - keep 1024).
        Host-permuted idx list lands tiles directly in PM layout.
      * remaining groups via PE one-hot matmul: token row broadcast with a
        K=1 outer product, one-hot built by is_equal vs per-partition iota,
        3 vocab-chunk matmuls (K=128/128/16) accumulate table rows in PSUM,
        scalar engine converts PSUM f32 -> SBUF bf16.
  - ADDR_KEY one-hot + MEM_STORE injection computed on-chip (scans on DVE,
    copy_predicated patches), identical for both gather paths.
  - Output written back with 8KB-contiguous DMA runs via the sync HWDGE queue.
"""

import sys
import numpy as np

for _p in ("/opt/trn_rl_repo",):
    if _p not in sys.path:
        sys.path.insert(0, _p)

# ---- problem constants (hardcoded per contract) ----
B, S, D, V = 32, 8192, 512, 272
NCORES = 8
RPC = B // NCORES          # rows (batch) per core = 4
P = 128                    # partitions
CPR = S // P               # columns per row in partition-major layout = 64
CTILE = 8                  # tile width in columns (CTILE*128 = 1024 tokens)
NTOK = RPC * S             # tokens per core = 32768
NG = NTOK // (P * CTILE)   # groups per core = 32
NDG = 23                   # groups gathered via dma_gather; rest via PE
IDXW = NDG * P * CTILE // 16
ADDR_KEY = 206
MEM_STORE = 455

_CACHE = {}


def _build(mhe: int):
    from concourse import bass, bacc, mybir, tile

    f32 = mybir.dt.float32
    f16 = mybir.dt.float16
    bf16 = mybir.dt.bfloat16
    i32 = mybir.dt.int32
    i16 = mybir.dt.int16
    u8 = mybir.dt.uint8
    Alu = mybir.AluOpType

    nc = bacc.Bacc(None, num_swdge_queues=4)
    tok_d = nc.declare_dram_parameter("tok", [RPC, S], i32, isOutput=False)
    idx_d = nc.declare_dram_parameter("idx", [P, IDXW], i16, isOutput=False)
    tokr_d = nc.declare_dram_parameter("tokr", [1, NTOK], f16, isOutput=False)
    tab_d = nc.declare_dram_parameter("table", [V, D], bf16, isOutput=False)
    out_d = nc.declare_dram_parameter("out", [RPC, S, D], bf16, isOutput=True)

    with tile.TileContext(nc) as tc:
        with tc.tile_pool(name="const", bufs=1) as constp, \
             tc.tile_pool(name="pre", bufs=1) as pre, \
             tc.tile_pool(name="dramp", bufs=1, space="DRAM") as dramp, \
             tc.tile_pool(name="mainp", bufs=4) as mainp, \
             tc.tile_pool(name="condp", bufs=3) as condp, \
             tc.tile_pool(name="ohp", bufs=3) as ohp, \
             tc.tile_pool(name="bcp", bufs=2, space="PSUM") as bcp, \
             tc.tile_pool(name="outp", bufs=3, space="PSUM") as outp:

            # ---------------- constants ----------------
            iota48_i = constp.tile([P, CTILE, 3, 16], i32)
            nc.gpsimd.iota(iota48_i[:], pattern=[[0, CTILE], [0, 3], [1, 16]],
                           base=0, channel_multiplier=0)
            iota48 = constp.tile([P, CTILE, 3, 16], f32)
            nc.vector.tensor_copy(iota48[:], iota48_i[:])

            # padded to 64 in the last dim so [:, :, 0:48] slices keep a
            # 3-D access pattern matching the strided x[...] views
            ones48 = constp.tile([P, CTILE, 64], bf16)
            nc.vector.memset(ones48[:], 1.0)

            pos_i = constp.tile([P, RPC, CPR], i32)   # pos = 64*p + c (per row)
            nc.gpsimd.iota(pos_i[:], pattern=[[0, RPC], [1, CPR]], base=0,
                           channel_multiplier=CPR)
            pos_f = constp.tile([P, RPC, CPR], f32)
            nc.vector.tensor_copy(pos_f[:], pos_i[:])

            # PE-gather constants: table chunks, ones row, per-chunk iota cols
            tab1 = constp.tile([P, D], bf16)
            nc.sync.dma_start(out=tab1[:], in_=tab_d[0:128, :])
            tab2 = constp.tile([P, D], bf16)
            nc.sync.dma_start(out=tab2[:], in_=tab_d[128:256, :])
            tab3 = constp.tile([16, D], bf16)
            nc.sync.dma_start(out=tab3[:], in_=tab_d[256:272, :])
            tabs = (tab1, tab2, tab3)

            ones1 = constp.tile([1, P], f16)
            nc.vector.memset(ones1[:], 1.0)

            iotav_i = constp.tile([P, 3], i32)        # p + 128k
            nc.gpsimd.iota(iotav_i[:], pattern=[[128, 3]], base=0,
                           channel_multiplier=1)
            iotav = constp.tile([P, 3], f32)
            nc.vector.tensor_copy(iotav[:], iotav_i[:])

            tokr_sb = constp.tile([1, NTOK], f16)
            nc.sync.dma_start(out=tokr_sb[:], in_=tokr_d[:])

            # ---------------- token / idx load ----------------
            tok_i = pre.tile([P, RPC, CPR], i32)
            nc.sync.dma_start(out=tok_i[:],
                              in_=tok_d[:].rearrange("r (p c) -> p r c", p=P))
            tok_f = pre.tile([P, RPC, CPR], f32)
            nc.vector.tensor_copy(tok_f[:], tok_i[:])

            idx_sb = pre.tile([P, IDXW], i16)
            nc.sync.dma_start(out=idx_sb[:], in_=idx_d[:])

            # ---------------- scan inputs ----------------
            posp1 = pre.tile([P, RPC, CPR], f32)
            nc.vector.tensor_scalar(posp1[:], pos_f[:], 1.0, None, Alu.add)
            posm1 = pre.tile([P, RPC, CPR], f32)
            nc.vector.tensor_scalar(posm1[:], pos_f[:], 1.0, None, Alu.subtract)

            # v0 = (tok==256)*(pos+1) - 1   (CODE_START candidate positions)
            v0 = pre.tile([P, RPC, CPR], f32)
            nc.vector.scalar_tensor_tensor(v0[:], tok_f[:], 256.0, posp1[:],
                                           Alu.is_equal, Alu.mult)
            nc.vector.tensor_scalar(v0[:], v0[:], 1.0, None, Alu.subtract)

            # v1 = (tok==257)  (CODE_END seen)
            v1 = pre.tile([P, RPC, CPR], f32)
            nc.vector.tensor_scalar(v1[:], tok_f[:], 257.0, None, Alu.is_equal)

            cs = pre.tile([P, RPC, CPR], f32)
            ce = pre.tile([P, RPC, CPR], f32)

            # --- level 1: within-partition prefix max over 64-token chunks ---
            loc_cs = pre.tile([P, RPC, CPR], f32)
            loc_ce = pre.tile([P, RPC, CPR], f32)
            for r in range(RPC):
                nc.vector.tensor_tensor_scan(loc_cs[:, r, :], v0[:, r, :],
                                             v0[:, r, :], -1.0,
                                             Alu.max, Alu.bypass)
                nc.vector.tensor_tensor_scan(loc_ce[:, r, :], v1[:, r, :],
                                             v1[:, r, :], 0.0,
                                             Alu.max, Alu.bypass)

            # --- level 2: exclusive prefix max across partitions (chunks) ---
            # Collect the 8 per-partition chunk-final columns (cs rows 0-3,
            # ce rows 4-7), transpose [128, 8] -> [8, 128] via a tiny DRAM
            # round-trip, scan along the free dim, shift for exclusivity,
            # transpose back.
            NS = 2 * RPC
            f8 = pre.tile([P, NS], f32)
            for r in range(RPC):
                nc.vector.tensor_copy(f8[:, r:r + 1],
                                      loc_cs[:, r, CPR - 1:CPR])
                nc.vector.tensor_copy(f8[:, RPC + r:RPC + r + 1],
                                      loc_ce[:, r, CPR - 1:CPR])
            f8_d = dramp.tile([P, NS], f32)
            nc.sync.dma_start(out=f8_d[:], in_=f8[:])
            f8t = pre.tile([NS, P], f32)
            nc.sync.dma_start(out=f8t[:], in_=f8_d[:].rearrange("p j -> j p"))
            p8 = pre.tile([NS, P], f32)
            nc.vector.tensor_tensor_scan(p8[:], f8t[:], f8t[:], -1e30,
                                         Alu.max, Alu.bypass)
            e8t = pre.tile([NS, P], f32)
            # -1 is a neutral carry for both scans (cs values >= -1, ce >= 0)
            nc.vector.memset(e8t[:, 0:1], -1.0)
            nc.vector.tensor_copy(e8t[:, 1:P], p8[:, 0:P - 1])
            e8_d = dramp.tile([NS, P], f32)
            nc.sync.dma_start(out=e8_d[:], in_=e8t[:])
            e8 = pre.tile([P, NS], f32)
            nc.sync.dma_start(out=e8[:], in_=e8_d[:].rearrange("j p -> p j"))

            # --- combine ---
            for r in range(RPC):
                nc.vector.tensor_scalar(cs[:, r, :], loc_cs[:, r, :],
                                        e8[:, r:r + 1], None, Alu.max)
                nc.vector.tensor_scalar(ce[:, r, :], loc_ce[:, r, :],
                                        e8[:, RPC + r:RPC + r + 1], None,
                                        Alu.max)

            # ---------------- per-token derived values ----------------
            # mask = (cs >= 0) & (ce == 0) & (tok < 256)
            m3 = pre.tile([P, RPC, CPR], f32)
            nc.vector.tensor_scalar(m3[:], tok_f[:], 255.5, None, Alu.is_lt)
            m23 = pre.tile([P, RPC, CPR], f32)
            nc.vector.scalar_tensor_tensor(m23[:], ce[:], 0.5, m3[:],
                                           Alu.is_lt, Alu.mult)
            mask = pre.tile([P, RPC, CPR], f32)
            nc.vector.scalar_tensor_tensor(mask[:], cs[:], 0.0, m23[:],
                                           Alu.is_ge, Alu.mult)

            # seq_pos = max(pos - 1 - cs, 0)
            sp = pre.tile([P, RPC, CPR], f32)
            nc.vector.scalar_tensor_tensor(sp[:], cs[:], -1.0, posm1[:],
                                           Alu.mult, Alu.add)
            nc.vector.tensor_scalar(sp[:], sp[:], 0.0, None, Alu.max)

            # q = floor(sp / 5), robust to cast rounding mode:
            #   y = sp*0.2 ; q0 = int(y) ; q = q0 - (y - float(q0) < 0)
            y = pre.tile([P, RPC, CPR], f32)
            nc.vector.tensor_scalar(y[:], sp[:], 0.2, None, Alu.mult)
            q_i = pre.tile([P, RPC, CPR], i32)
            nc.vector.tensor_copy(q_i[:], y[:])
            q_f = pre.tile([P, RPC, CPR], f32)
            nc.vector.tensor_copy(q_f[:], q_i[:])
            corr = pre.tile([P, RPC, CPR], f32)
            nc.vector.tensor_tensor(corr[:], y[:], q_f[:], Alu.subtract)
            nc.vector.tensor_scalar(corr[:], corr[:], 0.0, None, Alu.is_lt)
            nc.vector.tensor_tensor(q_f[:], q_f[:], corr[:], Alu.subtract)

            # addr = sp + 3*q  (int32)
            sp_i = pre.tile([P, RPC, CPR], i32)
            nc.vector.tensor_copy(sp_i[:], sp[:])
            q_i2 = pre.tile([P, RPC, CPR], i32)
            nc.vector.tensor_copy(q_i2[:], q_f[:])
            q3 = pre.tile([P, RPC, CPR], i32)
            nc.vector.tensor_scalar(q3[:], q_i2[:], 1, None, Alu.logical_shift_left)
            nc.vector.tensor_tensor(q3[:], q3[:], q_i2[:], Alu.add)
            addr = pre.tile([P, RPC, CPR], i32)
            nc.vector.tensor_tensor(addr[:], sp_i[:], q3[:], Alu.add)

            # nibbles
            lo_i = pre.tile([P, RPC, CPR], i32)
            nc.vector.tensor_scalar(lo_i[:], addr[:], 15, None, Alu.bitwise_and)
            hi_i = pre.tile([P, RPC, CPR], i32)
            nc.vector.tensor_scalar(hi_i[:], addr[:], 4, 15,
                                    Alu.logical_shift_right, Alu.bitwise_and)
            top_i = pre.tile([P, RPC, CPR], i32)
            nc.vector.tensor_scalar(top_i[:], addr[:], 8, 15,
                                    Alu.logical_shift_right, Alu.bitwise_and)
            # masked nibbles: nib_m = nib + 16*(1-mask) -- unmasked tokens
            # get an out-of-range value (>=16) so the iota 0..15 compare in
            # the cond build never fires; kills the separate mask multiply.
            # (single allocation: same-call-site tiles alias in a bufs=1 pool)
            nm3 = pre.tile([P, 3, RPC, CPR], f32)
            for b, src_i in enumerate((lo_i, hi_i, top_i)):
                nc.vector.tensor_copy(nm3[:, b], src_i[:])
            nc.vector.tensor_scalar(nm3[:], nm3[:], 16.0, None, Alu.add)
            for b in range(3):
                nc.vector.scalar_tensor_tensor(nm3[:, b], mask[:], -16.0,
                                               nm3[:, b], Alu.mult, Alu.add)

            # cond2 = (tok == 258) & (pos < mem_history_end)
            m5 = pre.tile([P, RPC, CPR], f32)
            nc.vector.tensor_scalar(m5[:], pos_f[:], float(mhe), None, Alu.is_lt)
            cond2 = pre.tile([P, RPC, CPR], u8)
            nc.vector.scalar_tensor_tensor(cond2[:], tok_f[:], 258.0, m5[:],
                                           Alu.is_equal, Alu.mult)

            # ---------------- main gather + patch + store loop ----------------
            out_v = out_d[:].rearrange("r (p c) d -> r p c d", p=P)
            NI = P * CTILE                 # tokens per group = 1024

            def patch_and_store(x, r, csl):
                cond = condp.tile([P, CTILE, 64], u8, tag="cond")
                for b in range(3):
                    nc.vector.tensor_tensor(
                        cond[:, :, 16 * b:16 * (b + 1)],
                        iota48[:, :, b, :],
                        nm3[:, b, r, csl].to_broadcast([P, CTILE, 16]),
                        Alu.is_equal)
                nc.vector.copy_predicated(
                    out=x[:, :, ADDR_KEY:ADDR_KEY + 48],
                    mask=cond[:, :, 0:48], data=ones48[:, :, 0:48])
                nc.vector.copy_predicated(
                    out=x[:, :, MEM_STORE],
                    mask=cond2[:, r, csl], data=ones48[:, :, 0])
                nc.sync.dma_start(out=out_v[r, :, csl, :], in_=x[:])

            # interleave dma_gather groups among PE groups so the Pool
            # gathers overlap PE work instead of serializing ahead of it
            dma_groups = list(range(NDG))
            pe_groups = list(range(NDG, NG))
            order = []
            di, pi = 0.0, 0
            ratio = len(pe_groups) / NDG
            for g in range(NG):
                if pi < len(pe_groups) and (di >= NDG or (pi + 1) / (di + 1) <= ratio):
                    order.append(pe_groups[pi]); pi += 1
                else:
                    order.append(dma_groups[int(di)]); di += 1

            for g in order:
                r, t = divmod(g, CPR // CTILE)
                c0 = t * CTILE
                csl = slice(c0, c0 + CTILE)
                x = mainp.tile([P, CTILE, D], bf16, tag="x")
                if g < NDG:
                    nc.gpsimd.dma_gather(
                        out_ap=x[:],
                        in_ap=tab_d[:],
                        idxs_ap=idx_sb[:, g * (NI // 16):(g + 1) * (NI // 16)],
                        num_idxs=NI,
                        num_idxs_reg=NI,
                        elem_size=D,
                        queue_num=g % 4,
                    )
                else:
                    for u in range(2):          # half-group = 4 columns
                        q0 = g * CTILE + u * 4  # global column index base
                        bc = bcp.tile([P, 4 * P], f32, tag="bc")
                        nc.tensor.matmul(bc[:], ones1[:],
                                         tokr_sb[:, q0 * P:(q0 + 4) * P],
                                         start=True, stop=True)
                        oh = ohp.tile([P, 3, 4 * P], bf16, tag="oh")
                        # vocab-chunk one-hots (must be DVE: Pool can't read
                        # PSUM, ACT has no is_equal)
                        nc.vector.tensor_scalar(oh[:, 0, :], bc[:],
                                                iotav[:, 0:1], None, Alu.is_equal)
                        nc.vector.tensor_scalar(oh[:, 1, :], bc[:],
                                                iotav[:, 1:2], None, Alu.is_equal)
                        nc.vector.tensor_scalar(oh[0:16, 2, :], bc[0:16, :],
                                                iotav[0:16, 2:3], None,
                                                Alu.is_equal)
                        for v in range(2):      # 2 columns share one PSUM pair
                            xp = outp.tile([P, 2, D], f32, tag="xp")
                            for w in range(2):
                                jj = 2 * v + w
                                msl = slice(jj * P, (jj + 1) * P)
                                for k, tb in enumerate(tabs):
                                    kp = tb.shape[0]
                                    nc.tensor.matmul(xp[:, w, :],
                                                     oh[0:kp, k, msl], tb[:],
                                                     start=(k == 0),
                                                     stop=(k == 2))
                            nc.scalar.copy(
                                x[:, u * 4 + 2 * v:u * 4 + 2 * v + 2, :],
                                xp[:])
                patch_and_store(x, r, csl)
    nc.finalize()
    return nc


def _get_nc(mhe: int):
    if mhe not in _CACHE:
        _CACHE[mhe] = _build(mhe)
    return _CACHE[mhe]


def _make_idx(tok_core: np.ndarray) -> np.ndarray:
    """[128, IDXW] int16 idx tensor for the NDG dma_gather groups.

    dma_gather: dst[p, j, :] = table[I[j*128 + p]] with I[i] =
    idxs[i % 16, i // 16] (16-partition wrap, replicated 8x to 128
    partitions).  Group g covers PM columns (r, c): r = g // 8,
    c in [8*(g%8), 8*(g%8)+8); dst[p, j] must be token (r, 64p + c0 + j).
    """
    A = tok_core.reshape(RPC, P, CPR)          # A[r, p, c] = tok[r, 64p + c]
    cols = []
    for g in range(NDG):
        r, t = divmod(g, CPR // CTILE)
        blk = A[r, :, t * CTILE:(t + 1) * CTILE]      # [128, CTILE]
        I = np.ascontiguousarray(blk.T).reshape(-1)   # I[j*128 + p]
        W = np.ascontiguousarray(I.reshape(-1, 16).T)  # [16, NI/16]
        cols.append(np.tile(W, (8, 1)))               # [128, NI/16]
    return np.ascontiguousarray(np.concatenate(cols, axis=1).astype(np.int16))


def _make_tokr(tok_core: np.ndarray) -> np.ndarray:
    """[1, NTOK] f16 token row for the PE groups: tokr[q*128 + p] =
    tok[r, 64p + c] with q = r*64 + c (fp16 is exact for vocab < 2048)."""
    A = tok_core.reshape(RPC, P, CPR)
    return np.ascontiguousarray(
        A.transpose(0, 2, 1).reshape(1, NTOK).astype(np.float16))


def _in_maps(token_ids, embed_table):
    import ml_dtypes
    tok = np.asarray(token_ids)
    tab = np.asarray(embed_table, dtype=np.float32)
    assert tok.shape == (B, S) and tab.shape == (V, D)
    tok = np.ascontiguousarray(tok.astype(np.int32, copy=False))
    tab16 = np.ascontiguousarray(tab.astype(ml_dtypes.bfloat16))
    maps = []
    for c in range(NCORES):
        tok_core = tok[c * RPC:(c + 1) * RPC]
        maps.append({
            "tok": tok_core,
            "idx": _make_idx(tok_core),
            "tokr": _make_tokr(tok_core),
            "table": tab16,
        })
    return maps


def kernel(token_ids, embed_table, mem_history_end):
    from concourse.bass_utils import run_bass_kernel_spmd

    mhe = int(mem_history_end)
    nc = _get_nc(mhe)
    in_maps = _in_maps(token_ids, embed_table)
    res = run_bass_kernel_spmd(nc, in_maps, list(range(NCORES))).results
    out = np.concatenate([np.asarray(res[c]["out"]) for c in range(NCORES)],
                         axis=0)
    return out.reshape(B, S, D).astype(np.float32)
